# revision 8
# baseline (speedup 1.0000x reference)
"""Trainium2 Bass kernel for causal multi-head attention with RoPE.

Full-input contract: kernel(**inputs) takes the unsharded tensors and
returns the full [B, S, D] output. Internally the work is sharded over
8 NeuronCores: cores 0-3 compute batch 0, cores 4-7 batch 1; within a
batch group each core owns 4 of the 16 heads (tensor-parallel over
heads). Each core computes its partial output-projection contribution
[S, D]; the host sums the 4 partials per batch and adds the biases
that commute with attention (wo_b, and wv_b which passes through the
softmax untouched because attention weights sum to 1).

Matmuls run in float32r (hardware rounds operands to ~11 mantissa
bits, fp32 accumulate in PSUM) at 4x the fp32 rate.
"""

import os
import sys

sys.path.insert(0, "/opt/trn_rl_repo")

import numpy as np

B = 2
S = 2048
D = 2048
H = 16
DK = 128
N_CORES = 8
HPC = 4          # heads per core
E = HPC * DK     # 512: per-core slice of the model dim
AN = 256         # phase-A sequence chunk (moving free dim for Q/K)
SC = 512         # attention query chunk (moving free dim)
KO = D // 128    # contraction chunks for the projections
NJ = S // 128    # key chunks
NI = S // SC     # query chunks
ISQRT_DK = 1.0 / np.sqrt(DK)

_CACHE = {}

last_exec_time_ns = None
last_results = None


def _build_program():
    import concourse.mybir as mybir
    import concourse.tile as tile
    from concourse import bacc

    dt = mybir.dt
    F32 = dt.float32
    F32R = dt.float32r
    AF = mybir.ActivationFunctionType

    nc = bacc.Bacc(None, target_bir_lowering=False, debug=True)

    xT = nc.dram_tensor("xT", [D, S], F32R, kind="ExternalInput")
    wqT = nc.dram_tensor("wqT", [D, E], F32R, kind="ExternalInput")
    wkT = nc.dram_tensor("wkT", [D, E], F32R, kind="ExternalInput")
    wvT = nc.dram_tensor("wvT", [D, E], F32R, kind="ExternalInput")
    woT = nc.dram_tensor("woT", [E, D], F32R, kind="ExternalInput")
    bq = nc.dram_tensor("bq", [HPC, DK], F32, kind="ExternalInput")
    bk = nc.dram_tensor("bk", [HPC, DK], F32, kind="ExternalInput")
    cc2 = nc.dram_tensor("cc2", [DK, S], F32R, kind="ExternalInput")
    sss = nc.dram_tensor("sss", [DK, S], F32R, kind="ExternalInput")
    masks = nc.dram_tensor("masks", [HPC, 128, SC], F32R, kind="ExternalInput")
    ones = nc.dram_tensor("ones", [128, 128], F32R, kind="ExternalInput")
    out = nc.dram_tensor("out", [S, D], F32, kind="ExternalOutput")

    with tile.TileContext(nc) as tc:
        with (
            tc.tile_pool(name="dram", bufs=1, space="DRAM") as dpool,
            tc.tile_pool(name="const", bufs=1) as cpool,
        ):
            q_d = dpool.tile([HPC, DK, S], F32R, name="q_d")
            k_d = dpool.tile([HPC, DK, S], F32R, name="k_d")
            v_d = dpool.tile([HPC, S, DK], F32R, name="v_d")

            bq_sb = cpool.tile([DK, HPC], F32, name="bq_sb")
            nc.sync.dma_start(bq_sb[:], bq[:].rearrange("h d -> d h"))
            bk_sb = cpool.tile([DK, HPC], F32, name="bk_sb")
            nc.sync.dma_start(bk_sb[:], bk[:].rearrange("h d -> d h"))

            # ---------- Phase A: Q/K/V projections ----------
            with (
                tc.tile_pool(name="aw", bufs=1) as awpool,
                tc.tile_pool(name="ax", bufs=2) as axpool,
                tc.tile_pool(name="ast", bufs=3) as astpool,
                tc.tile_pool(name="aps", bufs=2, space="PSUM") as apspool,
            ):
                # first x chunk before the (larger) weight loads so the
                # tensor engine gets work as early as possible
                xn_pre = []
                for n in range(2):
                    xnp = axpool.tile([128, KO, AN], F32R, tag="xn", name=f"xn{n}")
                    nc.sync.dma_start(
                        xnp[:],
                        xT[:, n * AN : (n + 1) * AN].rearrange(
                            "(ko p) s -> p ko s", p=128
                        ),
                    )
                    xn_pre.append(xnp)
                wq_sb = awpool.tile([128, KO, E], F32R, name="wq_sb")
                nc.sync.dma_start(
                    wq_sb[:], wqT[:].rearrange("(ko p) m -> p ko m", p=128)
                )
                wk_sb = awpool.tile([128, KO, E], F32R, name="wk_sb")
                nc.sync.dma_start(
                    wk_sb[:], wkT[:].rearrange("(ko p) m -> p ko m", p=128)
                )
                wv_sb = awpool.tile([128, KO, E], F32R, name="wv_sb")
                nc.sync.dma_start(
                    wv_sb[:], wvT[:].rearrange("(ko p) m -> p ko m", p=128)
                )

                for n in range(S // AN):
                    if n < 2:
                        xn = xn_pre[n]
                    else:
                        xn = axpool.tile([128, KO, AN], F32R, tag="xn")
                        nc.sync.dma_start(
                            xn[:],
                            xT[:, n * AN : (n + 1) * AN].rearrange(
                                "(ko p) s -> p ko s", p=128
                            ),
                        )
                    # Q and K: out[d, s] with d on partitions (RoPE-friendly)
                    for wsb, bsb, dst in ((wq_sb, bq_sb, q_d), (wk_sb, bk_sb, k_d)):
                        for m in range(HPC):
                            pq = apspool.tile([128, AN], F32, tag="pqk")
                            for k in range(KO):
                                nc.tensor.matmul(
                                    pq[:],
                                    wsb[:, k, m * DK : (m + 1) * DK],
                                    xn[:, k, :],
                                    start=(k == 0),
                                    stop=(k == KO - 1),
                                )
                            st = astpool.tile([128, AN], F32R, tag="qkst")
                            nc.scalar.activation(
                                st[:], pq[:], AF.Identity, bias=bsb[:, m : m + 1]
                            )
                            nc.sync.dma_start(
                                dst[m, :, n * AN : (n + 1) * AN], st[:]
                            )
                    # V: out[s, d] with s on partitions (natural for P@V)
                    for jj in range(AN // 128):
                        pv = apspool.tile([128, E], F32, tag="pv")
                        for k in range(KO):
                            nc.tensor.matmul(
                                pv[:],
                                xn[:, k, jj * 128 : (jj + 1) * 128],
                                wv_sb[:, k, :],
                                start=(k == 0),
                                stop=(k == KO - 1),
                            )
                        sv = astpool.tile([128, E], F32R, tag="vst")
                        nc.vector.tensor_copy(sv[:], pv[:])
                        j0 = n * AN + jj * 128
                        for h0 in range(HPC):
                            nc.sync.dma_start(
                                v_d[h0, j0 : j0 + 128, :],
                                sv[:, h0 * DK : (h0 + 1) * DK],
                            )

            # ---------- Phase B: causal attention per head ----------
            bc_ctx = tc.tile_pool(name="bconst", bufs=1)
            bcpool = bc_ctx.__enter__()
            ao_ctx = tc.tile_pool(name="ao", bufs=1)
            aopool = ao_ctx.__enter__()
            cw_ctx = tc.tile_pool(name="cw", bufs=1)
            cwpool = cw_ctx.__enter__()
            mask_sb = bcpool.tile([128, HPC, SC], F32R, name="mask_sb")
            nc.sync.dma_start(mask_sb[:], masks[:].rearrange("t p c -> p t c"))
            ones_sb = bcpool.tile([128, 128], F32R, name="ones_sb")
            nc.sync.dma_start(ones_sb[:], ones[:])
            cc2_sb = bcpool.tile([DK, S], F32R, name="cc2_sb")
            nc.sync.dma_start(cc2_sb[:], cc2[:])
            sss_sb = bcpool.tile([DK, S], F32R, name="sss_sb")
            nc.sync.dma_start(sss_sb[:], sss[:])

            ao_tiles = []
            wo_sb = cwpool.tile([128, HPC, D], F32R, name="wo_sb")
            with (
                tc.tile_pool(name="bkv", bufs=2) as bkv,
                tc.tile_pool(name="brt", bufs=2) as brt,
                tc.tile_pool(name="bp", bufs=6) as bp,
                tc.tile_pool(name="bli", bufs=2) as bli,
                tc.tile_pool(name="bps_s", bufs=3, space="PSUM") as bps_s,
                tc.tile_pool(name="bps_o", bufs=2, space="PSUM") as bps_o,
                tc.tile_pool(name="bps_l", bufs=2, space="PSUM") as bps_l,
            ):

                def rope_slice(dst_full, src_dram, h0, si, tagp):
                    # d-rows are packed [even dims; odd dims] per head, so
                    # the rotate pairs are partition r <-> partition r+64;
                    # read the opposite half directly (cross-partition APs)
                    sl = slice(si * SC, (si + 1) * SC)
                    raw = brt.tile([DK, SC], F32R, tag=tagp + "raw", name="raw")
                    nc.sync.dma_start(raw[:], src_dram[h0][:, sl])
                    sw = brt.tile([DK, SC], F32R, tag=tagp + "sw", name="sw")
                    nc.gpsimd.tensor_copy(sw[0:64, :], raw[64:128, :])
                    nc.gpsimd.tensor_copy(sw[64:128, :], raw[0:64, :])
                    nc.vector.tensor_mul(sw[:], sw[:], sss_sb[:, sl])
                    nc.vector.tensor_mul(dst_full[:, sl], raw[:], cc2_sb[:, sl])
                    nc.vector.tensor_add(dst_full[:, sl], dst_full[:, sl], sw[:])

                for h0 in range(HPC):
                    ktr = bkv.tile([DK, S], F32R, tag="ktr")
                    for si in range(NI):
                        rope_slice(ktr, k_d, h0, si, "k")
                    qtr = bkv.tile([DK, S], F32R, tag="qtr")
                    for si in range(NI):
                        rope_slice(qtr, q_d, h0, si, "q")
                    vt = bkv.tile([128, NJ, DK], F32R, tag="vt")
                    for jc in range(NJ):
                        nc.sync.dma_start(
                            vt[:, jc], v_d[h0, jc * 128 : (jc + 1) * 128, :]
                        )
                    if h0 == 0:
                        # prefetch the output-projection weights during B
                        nc.sync.dma_start(
                            wo_sb[:],
                            woT[:].rearrange("(ec p) f -> p ec f", p=128),
                        )

                    ao_t = aopool.tile([DK, S], F32R, name=f"ao_{h0}")
                    ao_tiles.append(ao_t)

                    for ic in range(NI):
                        po = bps_o.tile([128, SC], F32, tag="po")
                        pl = bps_l.tile([128, SC], F32, tag="pl")
                        njc = 4 * ic + 4
                        i0 = ic * SC

                        def emit_pv(p, jc, cs):
                            nc.tensor.matmul(
                                po[:, cs:],
                                vt[:, jc],
                                p[:, cs:],
                                start=(jc == 0),
                                stop=(jc == njc - 1),
                            )
                            nc.tensor.matmul(
                                pl[:, cs:],
                                ones_sb[:],
                                p[:, cs:],
                                start=(jc == 0),
                                stop=(jc == njc - 1),
                            )

                        pending = []
                        for jc in range(njc):
                            t = jc - 4 * ic  # >=0 on the causal diagonal band
                            cs = 128 * t if t >= 0 else 0
                            ps = bps_s.tile([128, SC], F32, tag="ps")
                            nc.tensor.matmul(
                                ps[:, cs:],
                                ktr[:, jc * 128 : (jc + 1) * 128],
                                qtr[:, i0 + cs : i0 + SC],
                                start=True,
                                stop=True,
                            )
                            p = bp.tile([128, SC], F32R, tag="p")
                            nc.scalar.activation(
                                p[:, cs:], ps[:, cs:], AF.Exp, scale=float(ISQRT_DK)
                            )
                            if t >= 0:
                                nc.gpsimd.tensor_mul(
                                    p[:, cs : cs + 128],
                                    p[:, cs : cs + 128],
                                    mask_sb[:, t, cs : cs + 128],
                                )
                            # software pipeline: scores run up to two tiles
                            # ahead of the P@V / row-sum matmuls so the ACT
                            # exp latency stays off the tensor-engine path
                            pending.append((p, jc, cs))
                            if len(pending) > 2:
                                emit_pv(*pending.pop(0))
                        for it in pending:
                            emit_pv(*it)

                        li = bli.tile([128, SC], F32, tag="li")
                        nc.vector.reciprocal_approx_fast(li[:], pl[:])
                        nc.vector.tensor_mul(
                            ao_t[:, i0 : i0 + SC], po[:], li[:]
                        )

            # ---------- Phase C: output projection (partial sum) ----------
            with (
                tc.tile_pool(name="cst", bufs=4) as cst,
                tc.tile_pool(name="cps", bufs=4, space="PSUM") as cps,
            ):
                for ii in range(S // 128):
                    pcs = [
                        cps.tile([128, 512], F32, tag="pc", name=f"pc_{ii}_{fc}")
                        for fc in range(4)
                    ]
                    for ec in range(HPC):
                        for fc in range(4):
                            nc.tensor.matmul(
                                pcs[fc][:],
                                ao_tiles[ec][:, ii * 128 : (ii + 1) * 128],
                                wo_sb[:, ec, fc * 512 : (fc + 1) * 512],
                                start=(ec == 0),
                                stop=(ec == HPC - 1),
                            )
                    for fc in range(4):
                        ob = cst.tile([128, 512], F32, tag="ob")
                        nc.vector.tensor_copy(ob[:], pcs[fc][:])
                        nc.sync.dma_start(
                            out[ii * 128 : (ii + 1) * 128, fc * 512 : (fc + 1) * 512],
                            ob[:],
                        )

            cw_ctx.__exit__(None, None, None)
            ao_ctx.__exit__(None, None, None)
            bc_ctx.__exit__(None, None, None)

    nc.compile()
    return nc


def _rope_tables():
    inv_freq = 1.0 / (10000.0 ** (np.arange(0, DK, 2, dtype=np.float64) / DK))
    pos = np.arange(S, dtype=np.float64)
    freqs = pos[:, None] * inv_freq[None, :]  # [S, DK/2]
    cos_t = np.cos(freqs).T.astype(np.float32)  # [64, S]
    sin_t = np.sin(freqs).T.astype(np.float32)
    cc2 = np.ascontiguousarray(np.concatenate([cos_t, cos_t], axis=0))
    sss = np.ascontiguousarray(np.concatenate([-sin_t, sin_t], axis=0))
    return cc2, sss


def kernel(
    x, wq_w, wq_b, wk_w, wk_b, wv_w, wv_b, wo_w, wo_b
) -> np.ndarray:
    global last_exec_time_ns, last_results
    from concourse.bass_utils import run_bass_kernel_spmd

    if "nc" not in _CACHE:
        _CACHE["nc"] = _build_program()
    nc = _CACHE["nc"]

    x = np.asarray(x, dtype=np.float32)
    wq_w = np.asarray(wq_w, dtype=np.float32)
    wk_w = np.asarray(wk_w, dtype=np.float32)
    wv_w = np.asarray(wv_w, dtype=np.float32)
    wo_w = np.asarray(wo_w, dtype=np.float32)
    wq_b = np.asarray(wq_b, dtype=np.float32)
    wk_b = np.asarray(wk_b, dtype=np.float32)
    wv_b = np.asarray(wv_b, dtype=np.float32)
    wo_b = np.asarray(wo_b, dtype=np.float32)

    cc2, sss = _rope_tables()
    r_idx = np.arange(128)[:, None]
    c_idx = np.arange(SC)[None, :]
    masks = np.ascontiguousarray(
        np.stack(
            [(r_idx <= c_idx - t * 128).astype(np.float32) for t in range(HPC)]
        )
    )
    ones = np.ones((128, 128), dtype=np.float32)
    # within each head, pack d-rows as [even dims; odd dims]
    perm = np.concatenate([np.arange(0, DK, 2), np.arange(1, DK, 2)])

    xT_b = [np.ascontiguousarray(x[b].T) for b in range(B)]

    in_maps = []
    for c in range(N_CORES):
        b = c // (N_CORES // B)
        g = c % (N_CORES // B)
        es = g * E

        def pack_qk(w):
            rows = w[es : es + E]  # [E, D]
            blocks = [
                rows[h0 * DK : (h0 + 1) * DK][perm] for h0 in range(HPC)
            ]
            return np.ascontiguousarray(np.concatenate(blocks, axis=0).T)

        def pack_bias(bvec):
            sl = bvec[es : es + E].reshape(HPC, DK)
            return np.ascontiguousarray(sl[:, perm])

        in_maps.append(
            {
                "xT": xT_b[b],
                "wqT": pack_qk(wq_w),
                "wkT": pack_qk(wk_w),
                "wvT": np.ascontiguousarray(wv_w[es : es + E].T),
                "woT": np.ascontiguousarray(wo_w[:, es : es + E].T),
                "bq": pack_bias(wq_b),
                "bk": pack_bias(wk_b),
                "cc2": cc2,
                "sss": sss,
                "masks": masks,
                "ones": ones,
            }
        )

    trace = bool(os.environ.get("MHA_TRACE"))
    res = run_bass_kernel_spmd(
        nc, in_maps, list(range(N_CORES)), trace=trace
    )
    last_exec_time_ns = res.exec_time_ns
    last_results = res

    # host-side gather: sum partials per batch, add biases that commute
    # with attention (softmax rows sum to 1, so wv_b passes straight
    # through to the output projection)
    const_bias = wo_b + wo_w @ wv_b  # [D]
    out = np.empty((B, S, D), dtype=np.float32)
    gpb = N_CORES // B
    for b in range(B):
        acc = res.results[b * gpb]["out"].copy()
        for c in range(b * gpb + 1, (b + 1) * gpb):
            acc += res.results[c]["out"]
        out[b] = acc + const_bias[None, :]
    return out
